# revision 5
# baseline (speedup 1.0000x reference)
"""Trainium2 Bass kernel for nn_Attend (l2-dist attention, b=4 h=8 n=2048 d=64).

Reference math:
    sim = 2*scale*(q@k^T) - ||q||^2 - ||k||^2   (scale = d^-0.5)
    sim = where(mask_j, sim, -FLT_MAX)
    out = softmax_j(sim) @ v

Device strategy (8 cores, pure data/head parallel, no collectives):
  - (b, h) pairs flattened; core c handles b = c//2, heads 4*(c%2)..+4.
  - ||q||^2 is constant per softmax row -> dropped (softmax shift-invariant).
  - mask is per (b, j): ~50% of keys masked.  Host compacts k/v to the valid
    columns only (padded to a multiple of 128), halving all device work.
  - No row-max pass: logits + C stay comfortably inside fp32 exp range.
    Per-key bias (C - ||k_j||^2, or -1e30 for padding) is applied via the
    ACT engine's per-partition bias during the exp.
  - Layout: S^T = K_c @ Q^T computed with keys on partitions (fp16 matmul,
    fp32 PSUM accumulate), exp on ACT -> P^T in fp16, then out^acc[i,65] +=
    P^T_tile^T @ [V|1] (fp16) accumulated over key tiles in PSUM.  Column 64
    (the ones column) is the softmax denominator; DVE reciprocal+scale
    finishes the division.  Host pre-transposes/casts/pads everything so the
    device does zero layout work.
"""

import os
import sys

import numpy as np

for _p in ("/root/.axon_site/_ro/trn_rl_repo", "/opt/trn_rl_repo"):
    if os.path.isdir(_p) and _p not in sys.path:
        sys.path.append(_p)

from contextlib import ExitStack

import concourse.bacc as bacc
import concourse.tile as tile
from concourse import mybir
from concourse.bass_utils import run_bass_kernel_spmd

N_CORES = 8
N_I = 2048          # queries per head
D = 64
HEADS_PER_CORE = 4
C_SHIFT = 30.0      # logit shift; keeps exp inputs in a comfortable range
PAD_BIAS = -1e30    # exp() underflows to exactly 0

_PROGRAM_CACHE = {}


def _build_program(j_tiles: int):
    """Bass program for one core: 4 heads of compacted attention."""
    nc = bacc.Bacc("TRN2", target_bir_lowering=False, debug=False)
    jp = j_tiles * 128
    f16, f32 = mybir.dt.float16, mybir.dt.float32

    qT = nc.dram_tensor("qT", [2, 128, N_I], f16, kind="ExternalInput").ap()
    kT = nc.dram_tensor("kT", [2, 128, jp], f16, kind="ExternalInput").ap()
    vS = nc.dram_tensor("vS", [4, 128, j_tiles * 65], f16, kind="ExternalInput").ap()
    bias = nc.dram_tensor("bias", [4, 128, j_tiles], f32, kind="ExternalInput").ap()
    out = nc.dram_tensor("out", [4, 128, 16, 64], f32, kind="ExternalOutput").ap()

    with tile.TileContext(nc) as tc, ExitStack() as ctx:
        inp = ctx.enter_context(tc.tile_pool(name="inp", bufs=1))
        pp = ctx.enter_context(tc.tile_pool(name="pp", bufs=3))
        outp = ctx.enter_context(tc.tile_pool(name="outp", bufs=2))
        rp = ctx.enter_context(tc.tile_pool(name="rp", bufs=2))
        ps_st = ctx.enter_context(tc.tile_pool(name="ps_st", bufs=2, space="PSUM"))
        ps_acc = ctx.enter_context(tc.tile_pool(name="ps_acc", bufs=4, space="PSUM"))

        qT_t, kT_t = [], []
        for t in range(2):
            qt = inp.tile([128, N_I], f16, tag=f"q{t}")
            nc.sync.dma_start(qt[:], qT[t])
            qT_t.append(qt)
            kt = inp.tile([128, jp], f16, tag=f"k{t}")
            nc.sync.dma_start(kt[:], kT[t])
            kT_t.append(kt)
        vS_t, bias_t = [], []
        for hh in range(4):
            vt = inp.tile([128, j_tiles * 65], f16, tag=f"v{hh}")
            nc.sync.dma_start(vt[:], vS[hh])
            vS_t.append(vt)
            bt = inp.tile([128, j_tiles], f32, tag=f"b{hh}")
            nc.sync.dma_start(bt[:], bias[hh])
            bias_t.append(bt)

        for hh in range(HEADS_PER_CORE):
            th, ph = hh // 2, 64 * (hh % 2)
            osb = outp.tile([128, 16, 64], f32, tag="osb")
            for ih in range(2):
                accs = [
                    ps_acc.tile([128, 4, 65], f32, tag="acc", name=f"acc_{hh}_{ih}_{g}")
                    for g in range(2)
                ]
                for jt in range(j_tiles):
                    st = ps_st.tile([128, 1024], f32, tag="st")
                    for half in range(2):
                        i0 = ih * 1024 + half * 512
                        nc.tensor.matmul(
                            st[:, half * 512:(half + 1) * 512],
                            kT_t[th][ph:ph + 64, jt * 128:(jt + 1) * 128],
                            qT_t[th][ph:ph + 64, i0:i0 + 512],
                            start=True, stop=True,
                        )
                    pt = pp.tile([128, 1024], f16, tag="pt")
                    nc.scalar.activation(
                        pt[:], st[:], mybir.ActivationFunctionType.Exp,
                        bias=bias_t[hh][:, jt:jt + 1], scale=1.0,
                    )
                    for s in range(8):
                        # start=True lazily zeroes the WHOLE 2KB psum bank
                        # (pending-zero bits); only the first slice-matmul of
                        # each bank may carry it.  Later slices at jt==0 then
                        # overwrite their still-pending bytes.
                        nc.tensor.matmul(
                            accs[s // 4][:, s % 4, :],
                            pt[:, s * 128:(s + 1) * 128],
                            vS_t[hh][:, jt * 65:(jt + 1) * 65],
                            start=(jt == 0 and s % 4 == 0),
                            stop=(jt == j_tiles - 1 and s % 4 == 3),
                            skip_group_check=True,
                        )
                for g in range(2):
                    r = rp.tile([128, 4], f32, tag="r")
                    nc.vector.reciprocal(r[:], accs[g][:, :, 64])
                    for s in range(4):
                        t_idx = ih * 8 + g * 4 + s
                        nc.vector.tensor_scalar_mul(
                            osb[:, t_idx, :], accs[g][:, s, 0:64], r[:, s:s + 1]
                        )
            nc.sync.dma_start(out[hh], osb[:])

    nc.compile()
    return nc


def _get_program(j_tiles: int):
    if j_tiles not in _PROGRAM_CACHE:
        _PROGRAM_CACHE[j_tiles] = _build_program(j_tiles)
    return _PROGRAM_CACHE[j_tiles]


def _prepare_inputs(q, k, v, mask, j_tiles, idxs):
    """Host-side shard + compact + transpose + cast for each core."""
    b, h, n, d = q.shape
    scale = d ** -0.5
    jp = j_tiles * 128
    in_maps = []
    for c in range(N_CORES):
        bi = c // 2
        ix = idxs[bi]
        nv = len(ix)
        qT_np = np.zeros((2, 128, N_I), np.float16)
        kT_np = np.zeros((2, 128, jp), np.float16)
        vS_np = np.zeros((4, 128, j_tiles * 65), np.float16)
        bias_np = np.full((4, 128, j_tiles), PAD_BIAS, np.float32)
        for hh in range(4):
            hi = (c % 2) * 4 + hh
            th, ph = hh // 2, 64 * (hh % 2)
            qT_np[th, ph:ph + 64, :] = (2.0 * scale * q[bi, hi]).T.astype(np.float16)
            kc = k[bi, hi, ix, :]
            kT_np[th, ph:ph + 64, :nv] = kc.T.astype(np.float16)
            vc = v[bi, hi, ix, :]
            va = np.concatenate(
                [vc, np.ones((nv, 1), np.float32)], axis=1
            ).astype(np.float16)
            vfull = np.zeros((jp, 65), np.float16)
            vfull[:nv] = va
            vS_np[hh] = (
                vfull.reshape(j_tiles, 128, 65).transpose(1, 0, 2)
                .reshape(128, j_tiles * 65)
            )
            ksq = (kc.astype(np.float64) ** 2).sum(-1).astype(np.float32)
            bfull = np.full((jp,), PAD_BIAS, np.float32)
            bfull[:nv] = C_SHIFT - ksq
            bias_np[hh] = bfull.reshape(j_tiles, 128).T
        in_maps.append({"qT": qT_np, "kT": kT_np, "vS": vS_np, "bias": bias_np})
    return in_maps


def _install_profile_shim():
    """Bridge concourse's NTFF trace path to the in-container profiler.

    concourse expects `antenv.axon_hooks.{get,set}_axon_ntff_profile_hook`;
    this image's antenv stub lacks it.  Recreate the module and register the
    ctypes hook from trn_agent_boot.  Also neuter upload_artifacts (no cloud
    bucket in-container).
    """
    import types
    import contextlib

    try:
        import antenv
        if "antenv.axon_hooks" not in sys.modules:
            mod = types.ModuleType("antenv.axon_hooks")
            mod._hook = None

            def set_axon_ntff_profile_hook(h):
                mod._hook = h

            def get_axon_ntff_profile_hook():
                return mod._hook

            mod.set_axon_ntff_profile_hook = set_axon_ntff_profile_hook
            mod.get_axon_ntff_profile_hook = get_axon_ntff_profile_hook
            sys.modules["antenv.axon_hooks"] = mod
            antenv.axon_hooks = mod
        from antenv import axon_hooks
        if axon_hooks.get_axon_ntff_profile_hook() is None:
            from trn_agent_boot.trn_boot import _ntff_profile_via_ctypes
            axon_hooks.set_axon_ntff_profile_hook(
                _ntff_profile_via_ctypes("/opt/axon/libaxon_pjrt.so")
            )
        import concourse.bass_utils as bu
        bu.upload_artifacts = lambda d: str(d)
        return axon_hooks.get_axon_ntff_profile_hook() is not None
    except Exception as e:  # pragma: no cover - profiling is best-effort
        print(f"profile shim failed: {e}")
        return False


def kernel(q, k, v, mask, _profile=False, _trace_kwargs=None):
    q = np.asarray(q, dtype=np.float32)
    k = np.asarray(k, dtype=np.float32)
    v = np.asarray(v, dtype=np.float32)
    mask = np.asarray(mask)
    b, h, n, d = q.shape

    idxs = [np.nonzero(mask[bi])[0] for bi in range(b)]
    max_nv = max(max(len(ix) for ix in idxs), 1)
    j_tiles = -(-max_nv // 128)

    nc = _get_program(j_tiles)
    in_maps = _prepare_inputs(q, k, v, mask, j_tiles, idxs)

    kwargs = {}
    if _profile and _install_profile_shim():
        kwargs["trace"] = True
        if _trace_kwargs:
            kwargs["trace_kwargs"] = _trace_kwargs
    res = run_bass_kernel_spmd(nc, in_maps, list(range(N_CORES)), **kwargs)

    out = np.empty((b, h, n, d), np.float32)
    for c in range(N_CORES):
        o = res.results[c]["out"]  # [4, 128, 16, 64]
        bi = c // 2
        for hh in range(4):
            hi = (c % 2) * 4 + hh
            out[bi, hi] = o[hh].transpose(1, 0, 2).reshape(n, d)
    if _profile:
        return out, res
    return out
